# revision 17
# baseline (speedup 1.0000x reference)
"""2-layer GCN (GCNConv+relu x2, linear head) on 8 Trainium2 NeuronCores.

Strategy (graph/data parallel, per sharding hint):
  - Nodes sharded across 8 cores by id; edges partitioned by destination.
  - Per core, destination nodes are bin-packed into B_FIX blocks of <=BLK
    dsts such that each (block, source-window) holds <= KCOL*128 edges.
    This gives an SPMD-uniform program; only tensor data varies per core.
  - Layer 1 aggregates raw dinv-scaled x rows (W1 applied after
    aggregation, which commutes), gathered from a host-staged full-table
    of 256B bf16 rows -- no phase-A matmul and no first AllGather.
  - Layer 2 gathers dinv*(relu(conv1)@W2) rows from an AllGather'ed
    bf16 table (rows padded to 256B; pad bytes never consumed).
  - The 4 source-window gather streams are issued on 4 distinct SWDGE
    queues so all 8 GpSimd Q7 cores generate DMA descriptors in
    parallel (queue q is served by Q7 pair {2q, 2q+1}).
  - Scatter-to-destination is a PE matmul against a one-hot selection
    matrix S built with one bf16 is_equal DVE op per gather batch.
"""

import numpy as np

import concourse.bass as bass
import concourse.mybir as mybir
import concourse.tile as tile
from concourse import bacc
from concourse import bass_utils

import ml_dtypes

F32 = mybir.dt.float32
BF16 = mybir.dt.bfloat16
I16 = mybir.dt.int16
NP_BF16 = ml_dtypes.bfloat16


class Cfg:
    def __init__(self, n_nodes, in_feat, hidden, n_classes, n_cores, n_c,
                 blk, kcol, b_fix, nq, c_batch):
        self.N = n_nodes
        self.IN_FEAT = in_feat            # 128 == padded table row width
        self.HIDDEN = hidden
        self.N_CLASSES = n_classes
        self.NC = n_cores
        self.N_C = n_c                    # nodes per core (id // N_C)
        assert n_c * n_cores >= n_nodes
        self.BLK = blk                    # max dsts per block
        self.KCOL = kcol                  # columns per (block, stream)
        self.CAP = kcol * 128             # max edges per (block, stream)
        self.B_FIX = b_fix                # blocks per core (uniform)
        self.NQ = nq                      # source windows / gather streams
        self.SLOTS_C = b_fix * blk        # table slots per core
        assert self.SLOTS_C % 128 == 0
        self.NT = self.SLOTS_C // 128     # node tiles per core
        assert self.NT % 2 == 0
        self.TABLE_N = n_cores * self.SLOTS_C
        assert self.TABLE_N % nq == 0
        self.WIN = self.TABLE_N // nq     # table rows per source window
        assert self.WIN <= 32767          # int16 gather index range
        assert (n_cores % nq) == 0
        self.COLS_Q = b_fix * kcol        # gather columns per stream
        self.C_BATCH = c_batch            # columns per gather batch
        assert c_batch % kcol == 0 and self.COLS_Q % c_batch == 0
        self.N_BATCH = self.COLS_Q // c_batch
        self.BPB = c_batch // kcol        # blocks per batch
        assert self.BPB % 2 == 0          # block pairs never straddle batches
        self.IDXW = c_batch * 128 // 16   # idx free-dim cols per batch
        self.QS = self.SLOTS_C // nq      # local slots per table quarter
        assert self.QS % 128 == 0
        # hs-tile quarter boundaries align with block-pair tiles
        assert self.QS // 128 * nq == self.NT


CFG_FULL = Cfg(n_nodes=100000, in_feat=128, hidden=64, n_classes=16,
               n_cores=8, n_c=12544, blk=64, kcol=2, b_fix=224, nq=4,
               c_batch=28)


# ---------------------------------------------------------------------------
# Host-side preprocessing (sharding): all integer graph restructuring.
# ---------------------------------------------------------------------------

def preprocess(cfg, x, edge_index, W1, b1, W2, b2, Wl, bl):
    N, NC, N_C = cfg.N, cfg.NC, cfg.N_C
    src = np.asarray(edge_index[0]).astype(np.int64)
    dst = np.asarray(edge_index[1]).astype(np.int64)
    x = np.asarray(x, dtype=np.float32)

    deg = np.bincount(dst, minlength=N).astype(np.float32) + 1.0
    dinv = (1.0 / np.sqrt(deg)).astype(np.float32)

    # --- stream = table quarter of the SOURCE's slot. Assign quarters
    # round-robin by decreasing degree (per core) before packing so
    # per-edge streams are known and balanced. ---
    quarter = np.zeros(N, dtype=np.int64)
    for c in range(NC):
        lo, hi = c * N_C, min((c + 1) * N_C, N)
        order = np.argsort(-deg[lo:hi], kind="stable")
        qa = np.empty(hi - lo, dtype=np.int64)
        qa[order] = np.arange(hi - lo) % cfg.NQ
        quarter[lo:hi] = qa
    q_of = quarter[src]                      # stream of each edge

    # per-(node, q) incoming edge counts
    degq = np.bincount(dst * cfg.NQ + q_of, minlength=N * cfg.NQ)\
             .reshape(N, cfg.NQ)

    # --- per-(core, quarter) first-fit-decreasing packing into blocks ---
    BQ = cfg.B_FIX // cfg.NQ                 # blocks per quarter
    slot_of = np.full(NC * N_C, -1, dtype=np.int64)
    node_of_slot = np.full(cfg.TABLE_N, -1, dtype=np.int64)
    for c in range(NC):
        lo, hi = c * N_C, min((c + 1) * N_C, N)
        n_here = hi - lo
        if n_here <= 0:
            continue
        for qq in range(cfg.NQ):
            ids = lo + np.flatnonzero(quarter[lo:hi] == qq)
            if ids.size == 0:
                continue
            dq = degq[ids]
            order = np.argsort(-dq.max(axis=1), kind="stable")
            accs = np.zeros((BQ, cfg.NQ), dtype=np.int64)
            cnts = np.zeros(BQ, dtype=np.int64)
            nopen = 1
            for j in order:
                v = dq[j]
                fits = (cnts[:nopen] < cfg.BLK) & \
                       np.all(accs[:nopen] + v <= cfg.CAP, axis=1)
                w = np.flatnonzero(fits)
                if w.size == 0:
                    assert nopen < BQ, \
                        f"core {c} quarter {qq}: packing exceeds {BQ} blocks"
                    b = nopen
                    nopen += 1
                else:
                    b = int(w[0])
                g = int(ids[j])
                s = c * cfg.SLOTS_C + qq * cfg.QS + b * cfg.BLK + cnts[b]
                slot_of[g] = s
                node_of_slot[s] = g
                accs[b] += v
                cnts[b] += 1

    slot_of = slot_of[:N]

    # --- per-core edge streams ---
    e_core = dst // N_C
    s_slot = slot_of[src]
    d_slot_l = slot_of[dst] - e_core * cfg.SLOTS_C
    e_b = d_slot_l // cfg.BLK
    e_r = d_slot_l % cfg.BLK

    P_Q = cfg.B_FIX * cfg.CAP            # positions per stream
    idx_all = np.zeros((NC, cfg.NQ, P_Q), dtype=np.int16)
    dl_all = np.full((NC, cfg.NQ, P_Q), 255.0, dtype=np.float32)

    order2 = np.lexsort((e_b, q_of, e_core))
    es_c, eq_c, eb_c = e_core[order2], q_of[order2], e_b[order2]
    grp = (es_c * cfg.NQ + eq_c) * cfg.B_FIX + eb_c
    _, start_idx, cnt_grp = np.unique(grp, return_index=True,
                                      return_counts=True)
    rank = np.arange(grp.size) - np.repeat(start_idx, cnt_grp)
    assert rank.max(initial=0) < cfg.CAP
    pos = eb_c * cfg.CAP + rank
    # quarter-window index: src at core c, local slot lc -> c*QS + lc%QS
    ss = s_slot[order2]
    idx_val = ((ss // cfg.SLOTS_C) * cfg.QS
               + (ss % cfg.SLOTS_C) % cfg.QS).astype(np.int16)
    idx_all[es_c, eq_c, pos] = idx_val
    dl_all[es_c, eq_c, pos] = e_r[order2].astype(np.float32)

    # shared idx tile: stream q wrapped-16 into partition band
    # [32q, 32q+16), replicated to [32q+16, 32q+32) -- queue q's Q7 pair
    # reads exactly that band.
    idx_w = idx_all.reshape(NC, cfg.NQ, -1, 16).transpose(0, 1, 3, 2)
    idxsh = np.zeros((NC, 128, P_Q // 16), dtype=np.int16)
    for q in range(cfg.NQ):
        idxsh[:, 32 * q:32 * q + 16] = idx_w[:, q]
        idxsh[:, 32 * q + 16:32 * q + 32] = idx_w[:, q]
    # dstloc layout: position -> [pos%128, pos//128]
    dl_dev = np.ascontiguousarray(
        dl_all.reshape(NC, cfg.NQ, cfg.COLS_Q, 128).transpose(0, 1, 3, 2)
    ).astype(NP_BF16)

    # --- full dinv-scaled x table, 256B bf16 rows, shared by all cores.
    # Quarter-major layout: global slot s (core c, local lc, quarter
    # qq=lc//QS) lands at row qq*WIN + c*QS + lc%QS -- matching the
    # per-quarter AllGather output layout used for layer 2. ---
    valid = node_of_slot >= 0
    s_all = np.arange(cfg.TABLE_N)
    xrow = ((s_all % cfg.SLOTS_C) // cfg.QS) * cfg.WIN \
        + (s_all // cfg.SLOTS_C) * cfg.QS + (s_all % cfg.SLOTS_C) % cfg.QS
    xtab = np.zeros((cfg.TABLE_N, cfg.IN_FEAT), dtype=NP_BF16)
    xtab[xrow[valid]] = (x[node_of_slot[valid]]
                         * dinv[node_of_slot[valid]][:, None]).astype(NP_BF16)
    dinv_s = np.zeros(cfg.TABLE_N, dtype=np.float32)
    dinv_s[valid] = dinv[node_of_slot[valid]]

    W1 = np.asarray(W1, np.float32).astype(NP_BF16)
    W2 = np.asarray(W2, np.float32).astype(NP_BF16)
    Wl = np.asarray(Wl, np.float32).astype(NP_BF16)
    b1 = np.asarray(b1, np.float32)
    b2 = np.asarray(b2, np.float32)
    bl = np.asarray(bl, np.float32)

    iota64 = np.tile(np.arange(cfg.BLK, dtype=np.float32)[None, :],
                     (128, 1)).astype(NP_BF16)
    ident2 = np.concatenate([np.eye(cfg.HIDDEN), np.eye(cfg.HIDDEN)],
                            axis=0).astype(NP_BF16)

    # local-slot-ordered self rows (xtab is quarter-major, so re-gather)
    xloc = np.zeros((cfg.TABLE_N, cfg.IN_FEAT), dtype=NP_BF16)
    xloc[s_all] = xtab[xrow]

    in_maps = []
    for c in range(NC):
        sl = slice(c * cfg.SLOTS_C, (c + 1) * cfg.SLOTS_C)
        dv = dinv_s[sl]
        m = {
            "xtab": xtab,
            "xself": np.ascontiguousarray(xloc[sl].T),
            "w1": W1, "w2": W2, "wl": Wl,
            "b1c": b1.reshape(-1, 1), "b2c": b2.reshape(-1, 1),
            "blrep": np.tile(bl[None, :], (128, 1)),
            "dinvn": np.ascontiguousarray(dv.reshape(cfg.NT, 128).T),
            "dinvfm": np.tile(dv[None, :], (cfg.HIDDEN, 1)),
            "iota64": iota64,
            "ident2": ident2,
            "idxsh": idxsh[c],
        }
        for q in range(cfg.NQ):
            m[f"dl{q}"] = dl_dev[c, q]
        in_maps.append(m)

    return in_maps, node_of_slot


def assemble_output(cfg, results, node_of_slot):
    out = np.zeros((cfg.N, cfg.N_CLASSES), dtype=np.float32)
    for c, r in enumerate(results):
        lg = r["logits"].reshape(128, cfg.NT, cfg.N_CLASSES)
        sl = node_of_slot[c * cfg.SLOTS_C:(c + 1) * cfg.SLOTS_C]\
            .reshape(cfg.NT, 128)
        for t in range(cfg.NT):
            v = sl[t] >= 0
            out[sl[t][v]] = lg[v, t, :]
    return out


# ---------------------------------------------------------------------------
# Device program
# ---------------------------------------------------------------------------

def build_program(cfg):
    nc = bacc.Bacc("TRN2", target_bir_lowering=False, debug=False,
                   num_devices=cfg.NC, num_swdge_queues=cfg.NQ)
    H, NT, CB = cfg.HIDDEN, cfg.NT, cfg.C_BATCH

    xtab_d = nc.dram_tensor("xtab", [cfg.TABLE_N, cfg.IN_FEAT], BF16,
                            kind="ExternalInput")
    xself_d = nc.dram_tensor("xself", [128, cfg.SLOTS_C], BF16,
                             kind="ExternalInput")
    w1_d = nc.dram_tensor("w1", [cfg.IN_FEAT, H], BF16, kind="ExternalInput")
    w2_d = nc.dram_tensor("w2", [H, H], BF16, kind="ExternalInput")
    wl_d = nc.dram_tensor("wl", [H, cfg.N_CLASSES], BF16,
                          kind="ExternalInput")
    b1c_d = nc.dram_tensor("b1c", [H, 1], F32, kind="ExternalInput")
    b2c_d = nc.dram_tensor("b2c", [H, 1], F32, kind="ExternalInput")
    blrep_d = nc.dram_tensor("blrep", [128, cfg.N_CLASSES], F32,
                             kind="ExternalInput")
    dinvn_d = nc.dram_tensor("dinvn", [128, NT], F32, kind="ExternalInput")
    dinvfm_d = nc.dram_tensor("dinvfm", [H, cfg.SLOTS_C], F32,
                              kind="ExternalInput")
    iota_d = nc.dram_tensor("iota64", [128, cfg.BLK], BF16,
                            kind="ExternalInput")
    ident_d = nc.dram_tensor("ident2", [128, H], BF16, kind="ExternalInput")
    idx_d = nc.dram_tensor("idxsh", [128, cfg.B_FIX * cfg.CAP // 16], I16,
                           kind="ExternalInput")
    dl_d = [nc.dram_tensor(f"dl{q}", [128, cfg.COLS_Q], BF16,
                           kind="ExternalInput") for q in range(cfg.NQ)]
    logits_d = nc.dram_tensor("logits", [128, NT * cfg.N_CLASSES], F32,
                              kind="ExternalOutput")

    rg = [list(range(cfg.NC))]

    with tile.TileContext(nc) as tc:
        with tc.tile_pool(name="const", bufs=1) as cpool, \
             tc.tile_pool(name="dram", bufs=1, space="DRAM") as dpool, \
             tc.tile_pool(name="hp", bufs=3) as hpool:

            hsQ = [dpool.tile([cfg.QS, cfg.IN_FEAT], BF16, name=f"hsQ{q}",
                              tag=f"hs1q{q}") for q in range(cfg.NQ)]
            tabQ = [dpool.tile([cfg.WIN, cfg.IN_FEAT], BF16, name=f"tabQ{q}",
                               tag=f"tab2q{q}", addr_space="Shared")
                    for q in range(cfg.NQ)]

            def cload(dram, shape, dt, tag):
                t = cpool.tile(shape, dt, tag=tag)
                nc.sync.dma_start(out=t[:], in_=dram[:, :])
                return t

            w1_s = cload(w1_d, [cfg.IN_FEAT, H], BF16, "w1")
            w2_s = cload(w2_d, [H, H], BF16, "w2")
            wl_s = cload(wl_d, [H, cfg.N_CLASSES], BF16, "wl")
            b1c_s = cload(b1c_d, [H, 1], F32, "b1c")
            b2c_s = cload(b2c_d, [H, 1], F32, "b2c")
            blrep_s = cload(blrep_d, [128, cfg.N_CLASSES], F32, "blrep")
            dinvn_s = cload(dinvn_d, [128, NT], F32, "dinvn")
            iota_s = cload(iota_d, [128, cfg.BLK], BF16, "iota")
            ident_s = cload(ident_d, [128, H], BF16, "ident")
            xself_s = cload(xself_d, [128, cfg.SLOTS_C], BF16, "xself")

            self2_s = cpool.tile([128, NT * H], BF16, tag="self2")
            stageL_s = cpool.tile([128, NT * cfg.N_CLASSES], F32, tag="stgL")

            with tc.tile_pool(name="sp", bufs=2) as spool, \
                 tc.tile_pool(name="pp", bufs=2, space="PSUM") as pp, \
                 tc.tile_pool(name="pq", bufs=2, space="PSUM") as pq:

                def emit_cc(q):
                    nc.gpsimd.collective_compute(
                        "AllGather", mybir.AluOpType.bypass,
                        replica_groups=rg,
                        ins=[hsQ[q].opt()], outs=[tabQ[q].opt()])

                def conv_layer(layer):
                    # fire quarter-q AllGather once its hs rows (written by
                    # batches < (q+1)*N_BATCH//NQ) are a couple batches old
                    cc_at = {(q + 1) * (cfg.N_BATCH // cfg.NQ) + 2: q
                             for q in range(cfg.NQ - 1)}
                    pair = {}
                    for i in range(cfg.N_BATCH):
                        if layer == 1 and i in cc_at:
                            emit_cc(cc_at[i])
                        idx_t = spool.tile([128, cfg.IDXW], I16, tag="idx")
                        nc.sync.dma_start(
                            out=idx_t[:],
                            in_=idx_d[:, i * cfg.IDXW:(i + 1) * cfg.IDXW])
                        msgs, Ss = [], []
                        for q in range(cfg.NQ):
                            dl_t = spool.tile([128, CB], BF16, tag=f"dl{q}")
                            nc.sync.dma_start(
                                out=dl_t[:],
                                in_=dl_d[q][:, i * CB:(i + 1) * CB])
                            msg_t = spool.tile([128, CB, cfg.IN_FEAT], BF16,
                                               tag=f"msg{q}")
                            if layer == 1:
                                src_ap = xtab_d[q * cfg.WIN:(q + 1) * cfg.WIN,
                                                :]
                            else:
                                src_ap = tabQ[q][:, :]
                            nc.gpsimd.dma_gather(
                                out_ap=msg_t[:],
                                in_ap=src_ap,
                                idxs_ap=idx_t[:],
                                num_idxs=CB * 128,
                                num_idxs_reg=CB * 128,
                                elem_size=cfg.IN_FEAT, queue_num=q,
                                single_packet=False)
                            S_t = spool.tile([128, CB, cfg.BLK], BF16,
                                             tag=f"S{q}")
                            iota_bc = iota_s[:]\
                                .rearrange("p (c f) -> p c f", c=1)\
                                .to_broadcast([128, CB, cfg.BLK])
                            dl_bc = dl_t[:]\
                                .rearrange("p (c f) -> p c f", f=1)\
                                .to_broadcast([128, CB, cfg.BLK])
                            nc.vector.tensor_tensor(
                                out=S_t[:], in0=iota_bc, in1=dl_bc,
                                op=mybir.AluOpType.is_equal)
                            msgs.append(msg_t[:].rearrange("p c f -> p (c f)"))
                            Ss.append(S_t[:].rearrange("p c f -> p (c f)"))

                        dfm_t = spool.tile([H, cfg.BPB * cfg.BLK], F32,
                                           tag="dfm")
                        nc.sync.dma_start(
                            out=dfm_t[:],
                            in_=dinvfm_d[:, i * cfg.BPB * cfg.BLK:
                                         (i + 1) * cfg.BPB * cfg.BLK])

                        for bb in range(cfg.BPB):
                            b = i * cfg.BPB + bb
                            half = (b % 2) * H
                            t = b // 2
                            if layer == 1:
                                # aggregate raw x rows: [128 feat, BLK dst]
                                pfm = pp.tile([128, cfg.BLK], F32, tag="fm")
                                for q in range(cfg.NQ):
                                    for k in range(cfg.KCOL):
                                        lc = bb * cfg.KCOL + k
                                        first = (q == 0 and k == 0)
                                        last = (q == cfg.NQ - 1 and
                                                k == cfg.KCOL - 1)
                                        nc.tensor.matmul(
                                            out=pfm[:],
                                            lhsT=msgs[q][:, lc * cfg.IN_FEAT:
                                                         (lc + 1) *
                                                         cfg.IN_FEAT],
                                            rhs=Ss[q][:, lc * cfg.BLK:
                                                      (lc + 1) * cfg.BLK],
                                            start=first, stop=last)
                                # + self term, downcast for W1 matmul
                                aggc = hpool.tile([128, cfg.BLK], BF16,
                                                  tag="aggc")
                                nc.vector.tensor_tensor(
                                    out=aggc[:], in0=pfm[:],
                                    in1=xself_s[:, b * cfg.BLK:
                                                (b + 1) * cfg.BLK],
                                    op=mybir.AluOpType.add)
                                pfw = pp.tile([H, cfg.BLK], F32, tag="fw1")
                                nc.tensor.matmul(out=pfw[:], lhsT=w1_s[:],
                                                 rhs=aggc[:],
                                                 start=True, stop=True)
                                post_src, bc_s = pfw, b1c_s
                            else:
                                pfm = pp.tile([H, cfg.BLK], F32, tag="fm",
                                              name="pfm")
                                nc.tensor.matmul(
                                    out=pfm[:],
                                    lhsT=self2_s[half:half + H,
                                                 t * H:(t + 1) * H],
                                    rhs=ident_s[half:half + H, :],
                                    start=True, stop=False)
                                for q in range(cfg.NQ):
                                    for k in range(cfg.KCOL):
                                        lc = bb * cfg.KCOL + k
                                        last = (q == cfg.NQ - 1 and
                                                k == cfg.KCOL - 1)
                                        nc.tensor.matmul(
                                            out=pfm[:],
                                            lhsT=msgs[q][:, lc * cfg.IN_FEAT:
                                                         lc * cfg.IN_FEAT
                                                         + H],
                                            rhs=Ss[q][:, lc * cfg.BLK:
                                                      (lc + 1) * cfg.BLK],
                                            start=False, stop=last)
                                post_src, bc_s = pfm, b2c_s

                            h_t = hpool.tile([H, cfg.BLK], F32, tag="h")
                            nc.vector.tensor_tensor(
                                out=h_t[:], in0=post_src[:],
                                in1=dfm_t[:, bb * cfg.BLK:(bb + 1) * cfg.BLK],
                                op=mybir.AluOpType.mult)
                            hr_t = hpool.tile([H, cfg.BLK], BF16, tag="hr")
                            nc.scalar.activation(
                                out=hr_t[:], in_=h_t[:],
                                func=mybir.ActivationFunctionType.Relu,
                                bias=bc_s[:])
                            if layer == 1:
                                if b % 2 == 0:
                                    pair["p2"] = pq.tile([128, H], F32,
                                                         name="p2",
                                                         tag="pair")
                                p2 = pair["p2"]
                                nc.tensor.matmul(
                                    out=p2[half:half + H, :], lhsT=hr_t[:],
                                    rhs=w2_s[:], start=True, stop=True,
                                    tile_position=(0, half))
                                if b % 2 == 1:
                                    row2 = hpool.tile([128, H], BF16,
                                                      tag="hs1row")
                                    nc.vector.tensor_scalar_mul(
                                        out=row2[:], in0=p2[:],
                                        scalar1=dinvn_s[:, t:t + 1])
                                    qh, tl = divmod(t, cfg.QS // 128)
                                    nc.sync.dma_start(
                                        out=hsQ[qh][tl * 128:(tl + 1) * 128,
                                                    0:H],
                                        in_=row2[:])
                                    nc.vector.tensor_scalar_mul(
                                        out=self2_s[:, t * H:(t + 1) * H],
                                        in0=p2[:],
                                        scalar1=dinvn_s[:, t:t + 1])
                            else:
                                if b % 2 == 0:
                                    pair["pl"] = pq.tile([128, cfg.N_CLASSES],
                                                         F32, name="pl",
                                                         tag="pl")
                                pl = pair["pl"]
                                nc.tensor.matmul(
                                    out=pl[half:half + H, :], lhsT=hr_t[:],
                                    rhs=wl_s[:], start=True, stop=True,
                                    tile_position=(0, half))
                                if b % 2 == 1:
                                    nCL = cfg.N_CLASSES
                                    nc.vector.tensor_tensor(
                                        out=stageL_s[:, t * nCL:(t + 1) * nCL],
                                        in0=pl[:], in1=blrep_s[:],
                                        op=mybir.AluOpType.add)

                conv_layer(1)
                emit_cc(cfg.NQ - 1)
                conv_layer(2)

            nc.sync.dma_start(out=logits_d[:, :], in_=stageL_s[:])

    nc.compile()
    return nc


_PROGRAM_CACHE = {}


def get_program(cfg):
    key = id(cfg)
    if key not in _PROGRAM_CACHE:
        _PROGRAM_CACHE[key] = build_program(cfg)
    return _PROGRAM_CACHE[key]


def run(cfg, inputs, trace=False):
    in_maps, node_of_slot = preprocess(cfg, **inputs)
    nc = get_program(cfg)
    res = bass_utils.run_bass_kernel_spmd(
        nc, in_maps, core_ids=list(range(cfg.NC)), trace=trace)
    out = assemble_output(cfg, res.results, node_of_slot)
    return out, res


def kernel(**inputs) -> np.ndarray:
    out, _ = run(CFG_FULL, inputs)
    return out
